# revision 3
# baseline (speedup 1.0000x reference)
"""Butterfly permuter kernel for Trainium2 (8 NeuronCores, SPMD data-parallel).

The reference applies 10 butterfly rotation stages along the feature axis
(dim=1024) of x [16384, 1024].  Stages 0..7 (spans 2..256) compose into a
block-diagonal matrix A7 with four 256x256 blocks; stages 8 (span 512) and
9 (span 1024) are elementwise rotations pairing column c with c+256 (within
each 512-half) and c+512 respectively.  So instead of one dense 1024x1024
matmul (PE cost 131k cycles/core) we do:

  y = stage9( stage8( x @ A7 ) )     with A7 block-diagonal (PE 32.8k cyc)

Per core (2048 tokens), per rep:
  - gpsimd (SWDGE) casting DMA: x fp32 DRAM -> bf16 SBUF megatiles
  - PE-transpose each [128 tok, 128 dim] bf16 block (1 cyc/row) -> X^T
    blocks with the contraction dim on partitions; DVE evacuates
    PSUM->SBUF into per-group [128, kb-major x 256 tok] bf16 tiles
  - BD matmul: for each 128-col output block j, 2 accumulating bf16
    matmuls (K=128 each) -> PSUM [128 col, 256 tok] fp32 = Y7^T blocks
    (transposed layout); ScalarE evacuates into 8 SBUF tiles
    yt_j [128 col, 2048 tok] bf16
  - stages 8+9 on VectorE in transposed layout: cols are partitions, so
    the per-column cos/sin are per-partition scalars: 4 tensor_scalar
    (4x mode) + 2 tensor_tensor (2x mode) ops per block pair, bf16.
    Stage 8 pairs (0,2),(1,3),(4,6),(5,7); stage 9 pairs (j, j+4).
  - PE-transpose back to [tok, col] (bf16, 1 cyc/row), DVE/ScalarE
    evacuate PSUM->SBUF as bf16, HWDGE DMA out bf16 (host upcasts to
    fp32 in the unshard/gather step; the SBUF values are bf16-rounded
    either way, so this costs no accuracy, only halves write traffic)

HBM/core/rep: 8 MiB fp32 in + 4.2 MiB bf16 out = 12.6 MB @ 358 GB/s
= 35.1 us roofline (vs 46.9 us for fp32 out).  PE: 16.4k (fwd T) +
32.8k (matmul) + 16.4k (back T) = 65.5k cyc @ 2.4 GHz = 27.3 us.

Work is software-pipelined across reps: stages 8+9 of rep r-1 and the
back-transposes of rep r-1 execute during rep r's front end.
"""

import numpy as np

import concourse.bass as bass
import concourse.mybir as mybir
import concourse.tile as tile
from concourse import bacc
from concourse.bass_utils import run_bass_kernel_spmd

N_CORES = 8
DIM = 1024
NUM_STAGES = 10
N_TOKENS = 16384
TOK_PER_CORE = N_TOKENS // N_CORES  # 2048
SUB = 128  # tokens per subtile (partition dim)
N_SUB = TOK_PER_CORE // SUB  # 16 subtiles
GRP = 2  # subtiles per matmul group (256-token moving operand)
N_GRP = N_SUB // GRP  # 8 groups
IN_CHUNK = 4  # subtiles per input DMA (SWDGE cast DMA, 2 MiB DRAM-side)
N_IN = N_SUB // IN_CHUNK  # 4
OUT_CHUNK = 2  # subtiles per output DMA (0.5 MiB bf16)
N_OUT = N_SUB // OUT_CHUNK  # 8

F32 = mybir.dt.float32
BF16 = mybir.dt.bfloat16
MULT = mybir.AluOpType.mult
ADD = mybir.AluOpType.add
SUBTRACT = mybir.AluOpType.subtract

# Block-pair schedule for the two elementwise stages in transposed
# layout: (left block, right block, coef column pair).  Stage-8 pairs
# must all precede any stage-9 pair that shares a block; emitting them
# in this order (one per matmul group) satisfies that.
STAGE_PAIRS = [
    (0, 2, 0),  # stage 8, theta8[0:128]
    (1, 3, 1),  # stage 8, theta8[128:256]
    (4, 6, 2),  # stage 8, theta8[256:384]
    (5, 7, 3),  # stage 8, theta8[384:512]
    (0, 4, 4),  # stage 9, theta9[0:128]
    (1, 5, 5),  # stage 9, theta9[128:256]
    (2, 6, 6),  # stage 9, theta9[256:384]
    (3, 7, 7),  # stage 9, theta9[384:512]
]


def compose_transform(angles: np.ndarray, n_stages: int = NUM_STAGES) -> np.ndarray:
    """Compose the first n_stages butterfly stages (float64), y = x @ R."""
    y = np.eye(DIM, dtype=np.float64)
    a = np.asarray(angles, dtype=np.float64)
    for s in range(n_stages):
        span = 2 ** (s + 1)
        half = span // 2
        y = y.reshape(-1, DIM // span, span)
        left, right = y[..., :half], y[..., half:]
        th = a[s].reshape(1, DIM // span, half)
        c, sn = np.cos(th), np.sin(th)
        y = np.concatenate([c * left + sn * right, -sn * left + c * right], -1)
        y = y.reshape(-1, DIM)
    return y


def build_bass(reps: int = 1):
    """reps>1 repeats the whole pipeline in one NEFF (for marginal timing)."""
    nc = bacc.Bacc(None, target_bir_lowering=False)
    x = nc.dram_tensor("x", [TOK_PER_CORE, DIM], F32, kind="ExternalInput")
    # w: 16 stationary blocks [128 k-dims, 128 cols], slot j*2+k
    w = nc.dram_tensor("w", [128, 16 * 128], BF16, kind="ExternalInput")
    ident = nc.dram_tensor("ident", [128, 128], BF16, kind="ExternalInput")
    # coef: [:, 2p] = cos for pair p, [:, 2p+1] = sin (p=0..3 stage 8,
    # p=4..7 stage 9)
    coef = nc.dram_tensor("coef", [128, 16], F32, kind="ExternalInput")
    y = nc.dram_tensor("y", [TOK_PER_CORE, DIM], BF16, kind="ExternalOutput")

    with tile.TileContext(nc) as tc:
        with (
            tc.tile_pool(name="const", bufs=1) as const_pool,
            tc.tile_pool(name="xbf", bufs=3) as xbf_pool,
            tc.tile_pool(name="xt", bufs=3) as xt_pool,
            tc.tile_pool(name="yt", bufs=2) as yt_pool,
            tc.tile_pool(name="tmp", bufs=8) as tmp_pool,
            tc.tile_pool(name="yout", bufs=3) as yout_pool,
            tc.tile_pool(name="pst", bufs=3, space="PSUM") as pst_pool,
            tc.tile_pool(name="psy", bufs=3, space="PSUM") as psy_pool,
            tc.tile_pool(name="psf", bufs=2, space="PSUM") as psf_pool,
        ):
            ident_sb = const_pool.tile([128, 128], BF16, name="ident_sb")
            nc.sync.dma_start(ident_sb[:], ident[:])
            coef_sb = const_pool.tile([128, 16], F32, name="coef_sb")
            nc.sync.dma_start(coef_sb[:], coef[:])
            w_sb = const_pool.tile([128, 16 * 128], BF16, name="w_sb")
            # j-major so the first out-blocks' weights arrive first
            for j in range(8):
                nc.scalar.dma_start(
                    w_sb[:, j * 256 : (j + 1) * 256],
                    w[:, j * 256 : (j + 1) * 256],
                )

            def cs_ap(p):
                return (
                    coef_sb[:, 2 * p : 2 * p + 1],
                    coef_sb[:, 2 * p + 1 : 2 * p + 2],
                )

            # ---- per-rep emission helpers ------------------------------
            def load_chunk(ci):
                """SWDGE casting DMA: 4 subtiles of x fp32 -> bf16 SBUF."""
                xb = xbf_pool.tile([128, IN_CHUNK * DIM], BF16, name="xb",
                                   tag="xb")
                r0 = ci * IN_CHUNK * SUB
                nc.gpsimd.dma_start(
                    xb[:].rearrange("p (s c) -> p s c", c=DIM),
                    x[r0 : r0 + IN_CHUNK * SUB, :].rearrange(
                        "(s p) c -> p s c", p=128
                    ),
                )
                return xb

            def emit_transpose(s, xb, xt_g):
                """Transpose subtile s's 8 blocks; evac into xt_g (kb-major).

                xt_g free layout: kb(8) x half(2) x tok(128); this subtile
                fills half h = s % GRP.
                """
                xcol = (s % IN_CHUNK) * DIM
                h = s % GRP
                # one PSUM bank holds all 8 transposed blocks of the subtile
                ps_t = pst_pool.tile([128, 1024], BF16, name="ps_t", tag="ps_t")
                for kb in range(8):
                    nc.tensor.transpose(
                        ps_t[:, kb * 128 : (kb + 1) * 128],
                        xb[:, xcol + kb * 128 : xcol + (kb + 1) * 128],
                        ident_sb,
                    )
                xtv = xt_g[:].rearrange("p (kb half t) -> p half kb t",
                                        half=GRP, t=128)
                # bf16 PSUM->SBUF on DVE runs in 2x_1p mode (658 ns/subtile),
                # cheaper than ScalarE and keeps ACT off the
                # transpose->matmul critical chain.
                nc.vector.tensor_copy(xtv[:, h : h + 1, :, :], ps_t[:])

            def emit_group_mm(g, xt_g, yts):
                """8 output blocks x 2 accumulating K=128 matmuls (bf16).

                Col block j contracts only against the two k-blocks of its
                own 256-block (A7 is BD-256): kb in {2*(j//2), 2*(j//2)+1}.
                """
                for jp in range(4):  # two output blocks share one PSUM bank
                    ps_y = psy_pool.tile([128, 512], F32, name="ps_y", tag="ps_y")
                    for jh in range(2):
                        j = jp * 2 + jh
                        base_k = 2 * (j // 2)
                        for k in range(2):
                            kb = base_k + k
                            nc.tensor.matmul(
                                ps_y[:, jh * 256 : (jh + 1) * 256],
                                w_sb[:, (j * 2 + k) * 128 : (j * 2 + k + 1) * 128],
                                xt_g[:, kb * 256 : (kb + 1) * 256],
                                start=(k == 0),
                                stop=(k == 1),
                            )
                        nc.scalar.copy(
                            yts[j][:, g * 256 : (g + 1) * 256],
                            ps_y[:, jh * 256 : (jh + 1) * 256],
                        )

            def emit_stage_pair(yts, pi):
                """Transposed-layout rotation for STAGE_PAIRS[pi]: per-
                partition cos/sin scalars, bf16 (tensor_scalar 4x mode,
                tensor_tensor 2x mode)."""
                li, ri, p = STAGE_PAIRS[pi]
                c_ap, s_ap = cs_ap(p)
                l, r = yts[li], yts[ri]
                t1 = tmp_pool.tile([128, TOK_PER_CORE], BF16, name="t1",
                                   tag="t9")
                t2 = tmp_pool.tile([128, TOK_PER_CORE], BF16, name="t2",
                                   tag="t9")
                t3 = tmp_pool.tile([128, TOK_PER_CORE], BF16, name="t3",
                                   tag="t9")
                t4 = tmp_pool.tile([128, TOK_PER_CORE], BF16, name="t4",
                                   tag="t9")
                nc.vector.tensor_scalar(t1[:], l[:], c_ap, None, MULT)
                nc.vector.tensor_scalar(t3[:], l[:], s_ap, None, MULT)
                nc.vector.tensor_scalar(t2[:], r[:], s_ap, None, MULT)
                nc.vector.tensor_scalar(t4[:], r[:], c_ap, None, MULT)
                nc.vector.tensor_tensor(l[:], t1[:], t2[:], ADD)
                nc.vector.tensor_tensor(r[:], t4[:], t3[:], SUBTRACT)

            def emit_back(rep_yts):
                """Back-transpose + bf16 evac + output DMA."""
                y_sb = None
                for s in range(N_SUB):
                    if s % OUT_CHUNK == 0:
                        y_sb = yout_pool.tile(
                            [128, OUT_CHUNK * DIM], BF16, name="y_sb", tag="y_sb"
                        )
                    base = (s % OUT_CHUNK) * DIM
                    ps_f0 = psf_pool.tile([128, 512], BF16, name="ps_f0",
                                          tag="ps_f")
                    ps_f1 = psf_pool.tile([128, 512], BF16, name="ps_f1",
                                          tag="ps_f")
                    for j in range(8):
                        dst = ps_f0 if j < 4 else ps_f1
                        jcol = (j % 4) * 128
                        nc.tensor.transpose(
                            dst[:, jcol : jcol + 128],
                            rep_yts[j][:, s * 128 : (s + 1) * 128],
                            ident_sb,
                        )
                    # alternate the two bf16 evacs between DVE and ScalarE so
                    # the psf-bank handoff is paced by neither alone (GpSimd
                    # cannot access PSUM on TRN2)
                    nc.vector.tensor_copy(y_sb[:, base : base + 512], ps_f0[:])
                    nc.scalar.copy(y_sb[:, base + 512 : base + DIM], ps_f1[:])
                    if s % OUT_CHUNK == OUT_CHUNK - 1:
                        r0 = (s - OUT_CHUNK + 1) * SUB
                        nc.scalar.dma_start(
                            y[r0 : r0 + OUT_CHUNK * SUB, :].rearrange(
                                "(s p) c -> p s c", p=128
                            ),
                            y_sb[:].rearrange("p (s c) -> p s c", c=DIM),
                        )

            # ---- software pipeline across reps -------------------------
            # Rep r's front end (loads/transposes/matmuls) interleaves the
            # stage-8/9 pairs of rep r-1 on DVE (their inputs are long since
            # ready, so DVE never convoys the transpose->matmul chain), then
            # emits rep r-1's back end (back-transposes + evac + store).
            prev_yts = None
            for _rep in range(reps):
                yts = [
                    yt_pool.tile([128, TOK_PER_CORE], BF16, name=f"yt{j}",
                                 tag=f"yt{j}")
                    for j in range(8)
                ]
                # transposes run one group ahead of the matmuls so the PE
                # never waits on the DVE PSUM->SBUF evacuation of its own
                # transpose outputs
                xb = load_chunk(0)
                xt_tiles = [None] * N_GRP

                def emit_group_tr(g, xb):
                    xt_g = xt_pool.tile([128, 8 * GRP * 128], BF16,
                                        name="xt_g", tag="xt_g")
                    for si in range(GRP):
                        emit_transpose(g * GRP + si, xb, xt_g)
                    xt_tiles[g] = xt_g

                emit_group_tr(0, xb)
                for g in range(N_GRP):
                    nxt = g + 1
                    if nxt < N_GRP:
                        if nxt * GRP % IN_CHUNK == 0:
                            ci = nxt * GRP // IN_CHUNK
                            if ci < N_IN:
                                xb = load_chunk(ci)
                        emit_group_tr(nxt, xb)
                    emit_group_mm(g, xt_tiles[g], yts)
                    if prev_yts is not None:
                        emit_stage_pair(prev_yts, g)
                if prev_yts is not None:
                    emit_back(prev_yts)
                prev_yts = yts
            for pi in range(len(STAGE_PAIRS)):
                emit_stage_pair(prev_yts, pi)
            emit_back(prev_yts)
    nc.compile()
    return nc


_NC_CACHE = None


def _get_nc():
    global _NC_CACHE
    if _NC_CACHE is None:
        _NC_CACHE = build_bass()
    return _NC_CACHE


def make_core_inputs(x: np.ndarray, angles: np.ndarray) -> list[dict]:
    """Per-core input maps (shared by run() and bench.py)."""
    import ml_dtypes

    x = np.ascontiguousarray(np.asarray(x, dtype=np.float32))
    angles = np.asarray(angles, dtype=np.float64)
    A7 = compose_transform(angles, 8)
    w = np.empty((128, 16 * 128), dtype=np.float64)
    for j in range(8):
        base_k = 2 * (j // 2)
        for k in range(2):
            blk = A7[(base_k + k) * 128 : (base_k + k + 1) * 128,
                     j * 128 : (j + 1) * 128]
            w[:, (j * 2 + k) * 128 : (j * 2 + k + 1) * 128] = blk
    w = w.astype(ml_dtypes.bfloat16)
    th8, th9 = angles[8], angles[9]
    coef = np.empty((128, 16), dtype=np.float32)
    for p in range(4):
        coef[:, 2 * p] = np.cos(th8[p * 128 : (p + 1) * 128])
        coef[:, 2 * p + 1] = np.sin(th8[p * 128 : (p + 1) * 128])
        coef[:, 8 + 2 * p] = np.cos(th9[p * 128 : (p + 1) * 128])
        coef[:, 9 + 2 * p] = np.sin(th9[p * 128 : (p + 1) * 128])
    ident = np.eye(128, dtype=ml_dtypes.bfloat16)
    return [
        {
            "x": x[c * TOK_PER_CORE : (c + 1) * TOK_PER_CORE],
            "w": w,
            "ident": ident,
            "coef": coef,
        }
        for c in range(N_CORES)
    ]


def finalize_output(y: np.ndarray) -> np.ndarray:
    """Unshard-side dtype fixup: reference output is fp32 (the device
    writes bf16; the values are bf16-rounded in SBUF either way)."""
    return np.ascontiguousarray(y.astype(np.float32, copy=False))


def run(x: np.ndarray, angles: np.ndarray, trace: bool = False):
    """Run on 8 cores; returns (y_full, BassKernelResults)."""
    nc = _get_nc()
    in_maps = make_core_inputs(x, angles)
    res = run_bass_kernel_spmd(
        nc, in_maps, core_ids=list(range(N_CORES)), trace=trace
    )
    y = np.concatenate(
        [np.asarray(res.results[c]["y"]) for c in range(N_CORES)], axis=0
    )
    return finalize_output(y), res


def kernel(x: np.ndarray, angles: np.ndarray) -> np.ndarray:
    y, _ = run(x, angles, trace=False)
    return y


# revision 7
# speedup vs baseline: 1851.6869x; 1851.6869x over previous
"""Butterfly permuter kernel for Trainium2 (8 NeuronCores, SPMD data-parallel).

The reference applies 10 butterfly rotation stages along the feature axis
(dim=1024) of x [16384, 1024].  Stages 0..7 (spans 2..256) compose into a
block-diagonal matrix A7 with four 256x256 blocks; stages 8 (span 512) and
9 (span 1024) are elementwise rotations pairing column c with c+256 (within
each 512-half) and c+512 respectively:

  y = stage9( stage8( x @ A7 ) )     with A7 block-diagonal (PE 32.8k cyc)

Per core (2048 tokens), per rep:
  - gpsimd (SWDGE) casting DMA: x fp32 DRAM -> bf16 SBUF megatiles
  - PE-transpose each [128 tok, 128 dim] bf16 block (1 cyc/row) -> X^T
    blocks with the contraction dim on partitions; DVE evacuates
    PSUM->SBUF into per-group [128, kb-major x 512 tok] bf16 tiles
  - BD matmul (512-token moving operand): for each 128-col output block
    j, 2 accumulating bf16 matmuls (K=128 each, A7's own 256-block)
    -> PSUM [128 col, 512 tok] fp32 = Y7^T; ScalarE evacuates into 8
    SBUF tiles yt_j [128 col, 2048 tok] bf16
  - stage 8 on VectorE in transposed layout (cols are partitions, so
    cos/sin are per-partition scalars): 4 tensor_scalar (4x mode) +
    2 tensor_tensor (2x) per pair; pairs (0,2),(1,3),(4,6),(5,7)
  - stage 9 DEFERRED-ADD: only the 4 products per pair (tensor_scalar
    4x; sin negated on one side so both combines are adds); the adds
    happen for free in PSUM during the accumulating back-transpose
    (2 chained ident-matmuls per output block on PE)
  - DVE/ScalarE evacuate the fp32 PSUM result as bf16, HWDGE DMA out
    bf16 (host upcasts to fp32 in the unshard/gather step; the SBUF
    values are bf16-rounded either way, so this costs no accuracy,
    only halves write traffic)

Work is software-pipelined across reps: stage ops of rep r-1 and the
back path of rep r-1 execute during rep r's front end.
"""

import numpy as np

import concourse.bass as bass
import concourse.mybir as mybir
import concourse.tile as tile
from concourse import bacc
from concourse.bass_utils import run_bass_kernel_spmd

N_CORES = 8
DIM = 1024
NUM_STAGES = 10
N_TOKENS = 16384
TOK_PER_CORE = N_TOKENS // N_CORES  # 2048
SUB = 128  # tokens per subtile (partition dim)
N_SUB = TOK_PER_CORE // SUB  # 16 subtiles
GRP = 4  # subtiles per matmul group (512-token moving operand)
N_GRP = N_SUB // GRP  # 4 groups
IN_CHUNK = 4  # subtiles per input DMA (SWDGE cast DMA, 2 MiB DRAM-side)
N_IN = N_SUB // IN_CHUNK  # 4
OUT_CHUNK = 2  # subtiles per output DMA (0.5 MiB bf16)

F32 = mybir.dt.float32
BF16 = mybir.dt.bfloat16
MULT = mybir.AluOpType.mult
ADD = mybir.AluOpType.add
SUBTRACT = mybir.AluOpType.subtract

# Stage-8 block pairs (transposed layout): (left, right, coef col pair).
STAGE8_PAIRS = [(0, 2, 0), (1, 3, 1), (4, 6, 2), (5, 7, 3)]
# Stage-9 pairs: (j, j+4) with coef pair 4+j.
STAGE9_PAIRS = [(0, 4, 4), (1, 5, 5), (2, 6, 6), (3, 7, 7)]


def compose_transform(angles: np.ndarray, n_stages: int = NUM_STAGES) -> np.ndarray:
    """Compose the first n_stages butterfly stages (float64), y = x @ R."""
    y = np.eye(DIM, dtype=np.float64)
    a = np.asarray(angles, dtype=np.float64)
    for s in range(n_stages):
        span = 2 ** (s + 1)
        half = span // 2
        y = y.reshape(-1, DIM // span, span)
        left, right = y[..., :half], y[..., half:]
        th = a[s].reshape(1, DIM // span, half)
        c, sn = np.cos(th), np.sin(th)
        y = np.concatenate([c * left + sn * right, -sn * left + c * right], -1)
        y = y.reshape(-1, DIM)
    return y


def build_bass(reps: int = 1):
    """reps>1 repeats the whole pipeline in one NEFF (for marginal timing)."""
    nc = bacc.Bacc(None, target_bir_lowering=False)
    x = nc.dram_tensor("x", [TOK_PER_CORE, DIM], F32, kind="ExternalInput")
    # w: 16 stationary blocks [128 k-dims, 128 cols], slot j*2+k
    w = nc.dram_tensor("w", [128, 16 * 128], BF16, kind="ExternalInput")
    ident = nc.dram_tensor("ident", [128, 128], BF16, kind="ExternalInput")
    # coef: cols 2p=cos, 2p+1=sin (p<4 stage 8, p>=4 stage 9);
    # cols 16+p: negated sin (deferred stage-9 combine is add-only)
    coef = nc.dram_tensor("coef", [128, 24], F32, kind="ExternalInput")
    y = nc.dram_tensor("y", [TOK_PER_CORE, DIM], BF16, kind="ExternalOutput")

    with tile.TileContext(nc) as tc:
        with (
            tc.tile_pool(name="const", bufs=1) as const_pool,
            tc.tile_pool(name="xbf", bufs=3) as xbf_pool,
            tc.tile_pool(name="xt", bufs=3) as xt_pool,
            tc.tile_pool(name="yt", bufs=2) as yt_pool,
            tc.tile_pool(name="tmp", bufs=8) as tmp_pool,
            tc.tile_pool(name="prod", bufs=1) as prod_pool,
            tc.tile_pool(name="yout", bufs=3) as yout_pool,
            tc.tile_pool(name="pst", bufs=3, space="PSUM") as pst_pool,
            tc.tile_pool(name="psy", bufs=3, space="PSUM") as psy_pool,
            tc.tile_pool(name="psf", bufs=2, space="PSUM") as psf_pool,
        ):
            ident_sb = const_pool.tile([128, 128], BF16, name="ident_sb")
            nc.sync.dma_start(ident_sb[:], ident[:])
            coef_sb = const_pool.tile([128, 24], F32, name="coef_sb")
            nc.sync.dma_start(coef_sb[:], coef[:])
            w_sb = const_pool.tile([128, 16 * 128], BF16, name="w_sb")
            for j in range(8):
                nc.scalar.dma_start(
                    w_sb[:, j * 256 : (j + 1) * 256],
                    w[:, j * 256 : (j + 1) * 256],
                )

            def cs_ap(p, neg_sin=False):
                c = coef_sb[:, 2 * p : 2 * p + 1]
                s = (coef_sb[:, 16 + p : 17 + p] if neg_sin
                     else coef_sb[:, 2 * p + 1 : 2 * p + 2])
                return c, s

            # ---- per-rep emission helpers ------------------------------
            def load_chunk(ci):
                """SWDGE casting DMA: 4 subtiles of x fp32 -> bf16 SBUF."""
                xb = xbf_pool.tile([128, IN_CHUNK * DIM], BF16, name="xb",
                                   tag="xb")
                r0 = ci * IN_CHUNK * SUB
                nc.gpsimd.dma_start(
                    xb[:].rearrange("p (s c) -> p s c", c=DIM),
                    x[r0 : r0 + IN_CHUNK * SUB, :].rearrange(
                        "(s p) c -> p s c", p=128
                    ),
                )
                return xb

            def emit_transpose(s, xb, xt_g):
                """Transpose subtile s's 8 blocks; evac into xt_g (kb-major).

                xt_g free layout: kb(8) x quarter(GRP) x tok(128); this
                subtile fills quarter h = s % GRP.
                """
                xcol = (s % IN_CHUNK) * DIM
                h = s % GRP
                ps_t = pst_pool.tile([128, 1024], BF16, name="ps_t", tag="ps_t")
                for kb in range(8):
                    nc.tensor.transpose(
                        ps_t[:, kb * 128 : (kb + 1) * 128],
                        xb[:, xcol + kb * 128 : xcol + (kb + 1) * 128],
                        ident_sb,
                    )
                xtv = xt_g[:].rearrange("p (kb half t) -> p half kb t",
                                        half=GRP, t=128)
                # bf16 PSUM->SBUF on DVE runs in 2x_1p mode, cheaper than
                # ScalarE and keeps ACT off the transpose->matmul chain.
                nc.vector.tensor_copy(xtv[:, h : h + 1, :, :], ps_t[:])

            def emit_group_mm(g, xt_g, yts):
                """8 output blocks x 2 accumulating K=128 matmuls (bf16),
                512-token moving operand, one PSUM bank per block."""
                for j in range(8):
                    ps_y = psy_pool.tile([128, 512], F32, name="ps_y",
                                         tag="ps_y")
                    base_k = 2 * (j // 2)
                    for k in range(2):
                        kb = base_k + k
                        nc.tensor.matmul(
                            ps_y[:],
                            w_sb[:, (j * 2 + k) * 128 : (j * 2 + k + 1) * 128],
                            xt_g[:, kb * GRP * 128 : (kb + 1) * GRP * 128],
                            start=(k == 0),
                            stop=(k == 1),
                        )
                    nc.scalar.copy(
                        yts[j][:, g * GRP * 128 : (g + 1) * GRP * 128],
                        ps_y[:],
                    )

            def emit_stage8_pair(yts, pi):
                """Transposed-layout rotation for STAGE8_PAIRS[pi]."""
                li, ri, p = STAGE8_PAIRS[pi]
                c_ap, s_ap = cs_ap(p)
                l, r = yts[li], yts[ri]
                t1 = tmp_pool.tile([128, TOK_PER_CORE], BF16, name="t1", tag="t9")
                t2 = tmp_pool.tile([128, TOK_PER_CORE], BF16, name="t2", tag="t9")
                t3 = tmp_pool.tile([128, TOK_PER_CORE], BF16, name="t3", tag="t9")
                t4 = tmp_pool.tile([128, TOK_PER_CORE], BF16, name="t4", tag="t9")
                nc.vector.tensor_scalar(t1[:], l[:], c_ap, None, MULT)
                nc.vector.tensor_scalar(t3[:], l[:], s_ap, None, MULT)
                nc.vector.tensor_scalar(t2[:], r[:], s_ap, None, MULT)
                nc.vector.tensor_scalar(t4[:], r[:], c_ap, None, MULT)
                nc.vector.tensor_tensor(l[:], t1[:], t2[:], ADD)
                nc.vector.tensor_tensor(r[:], t4[:], t3[:], SUBTRACT)

            def emit_stage9_defer(yts, prods, pi):
                """Stage-9 pair (j, j+4): only the 4 products; the adds
                happen in PSUM during the accumulating back-transpose."""
                li, ri, p = STAGE9_PAIRS[pi]
                c_ap, s_ap = cs_ap(p)
                _, ns_ap = cs_ap(p, neg_sin=True)
                pb_l = prod_pool.tile([128, TOK_PER_CORE], BF16,
                                      name=f"pb{li}", tag=f"pb{li}")
                pb_r = prod_pool.tile([128, TOK_PER_CORE], BF16,
                                      name=f"pb{ri}", tag=f"pb{ri}")
                # sin-products first (read both blocks), then cos-products
                # overwrite the yts blocks in place
                nc.vector.tensor_scalar(pb_l[:], yts[ri][:], s_ap, None, MULT)
                nc.vector.tensor_scalar(pb_r[:], yts[li][:], ns_ap, None, MULT)
                nc.vector.tensor_scalar(yts[li][:], yts[li][:], c_ap, None, MULT)
                nc.vector.tensor_scalar(yts[ri][:], yts[ri][:], c_ap, None, MULT)
                prods[li] = (yts[li], pb_l)
                prods[ri] = (yts[ri], pb_r)

            def emit_back(rep_prods):
                """Accumulating back-transpose (the deferred stage-9 adds)
                + bf16 evac + output DMA."""
                y_sb = None
                for s in range(N_SUB):
                    if s % OUT_CHUNK == 0:
                        y_sb = yout_pool.tile(
                            [128, OUT_CHUNK * DIM], BF16, name="y_sb",
                            tag="y_sb")
                    base = (s % OUT_CHUNK) * DIM
                    ps_f0 = psf_pool.tile([128, 512], F32, name="ps_f0",
                                          tag="ps_f")
                    ps_f1 = psf_pool.tile([128, 512], F32, name="ps_f1",
                                          tag="ps_f")
                    for j in range(8):
                        dst = ps_f0 if j < 4 else ps_f1
                        jcol = (j % 4) * 128
                        pa, pb = rep_prods[j]
                        nc.tensor.matmul(
                            dst[:, jcol : jcol + 128],
                            pa[:, s * 128 : (s + 1) * 128],
                            ident_sb,
                            start=True, stop=False,
                        )
                        nc.tensor.matmul(
                            dst[:, jcol : jcol + 128],
                            pb[:, s * 128 : (s + 1) * 128],
                            ident_sb,
                            start=False, stop=True,
                        )
                    # alternate the two evacs between DVE and ScalarE
                    nc.vector.tensor_copy(y_sb[:, base : base + 512], ps_f0[:])
                    nc.scalar.copy(y_sb[:, base + 512 : base + DIM], ps_f1[:])
                    if s % OUT_CHUNK == OUT_CHUNK - 1:
                        r0 = (s - OUT_CHUNK + 1) * SUB
                        nc.scalar.dma_start(
                            y[r0 : r0 + OUT_CHUNK * SUB, :].rearrange(
                                "(s p) c -> p s c", p=128
                            ),
                            y_sb[:].rearrange("p (s c) -> p s c", c=DIM),
                        )

            # ---- software pipeline across reps -------------------------
            prev_yts = None
            prev_prods = None

            def emit_pairs_for(yts, prods, pi):
                if pi < 4:
                    emit_stage8_pair(yts, pi)
                else:
                    emit_stage9_defer(yts, prods, pi - 4)

            for _rep in range(reps):
                yts = [
                    yt_pool.tile([128, TOK_PER_CORE], BF16, name=f"yt{j}",
                                 tag=f"yt{j}")
                    for j in range(8)
                ]
                xb = load_chunk(0)
                xt_tiles = [None] * N_GRP

                def emit_group_tr(g, xb):
                    xt_g = xt_pool.tile([128, 8 * GRP * 128], BF16,
                                        name="xt_g", tag="xt_g")
                    for si in range(GRP):
                        emit_transpose(g * GRP + si, xb, xt_g)
                    xt_tiles[g] = xt_g

                emit_group_tr(0, xb)
                pairs_done = 0
                cur_prods = {}
                for g in range(N_GRP):
                    nxt = g + 1
                    if nxt < N_GRP:
                        if nxt * GRP % IN_CHUNK == 0:
                            ci = nxt * GRP // IN_CHUNK
                            if ci < N_IN:
                                xb = load_chunk(ci)
                        emit_group_tr(nxt, xb)
                    emit_group_mm(g, xt_tiles[g], yts)
                    if prev_yts is not None:
                        want = (g + 1) * 8 // N_GRP
                        while pairs_done < want:
                            emit_pairs_for(prev_yts, prev_prods, pairs_done)
                            pairs_done += 1
                if prev_yts is not None:
                    emit_back(prev_prods)
                prev_yts = yts
                prev_prods = cur_prods
            for pi in range(8):
                emit_pairs_for(prev_yts, prev_prods, pi)
            emit_back(prev_prods)
    nc.compile()
    return nc


_NC_CACHE = None


def _get_nc():
    global _NC_CACHE
    if _NC_CACHE is None:
        _NC_CACHE = build_bass()
    return _NC_CACHE


def make_core_inputs(x: np.ndarray, angles: np.ndarray) -> list[dict]:
    """Per-core input maps (shared by run() and bench.py)."""
    import ml_dtypes

    x = np.ascontiguousarray(np.asarray(x, dtype=np.float32))
    angles = np.asarray(angles, dtype=np.float64)
    A7 = compose_transform(angles, 8)
    w = np.empty((128, 16 * 128), dtype=np.float64)
    for j in range(8):
        base_k = 2 * (j // 2)
        for k in range(2):
            blk = A7[(base_k + k) * 128 : (base_k + k + 1) * 128,
                     j * 128 : (j + 1) * 128]
            w[:, (j * 2 + k) * 128 : (j * 2 + k + 1) * 128] = blk
    w = w.astype(ml_dtypes.bfloat16)
    th8, th9 = angles[8], angles[9]
    coef = np.zeros((128, 24), dtype=np.float32)
    for p in range(4):
        coef[:, 2 * p] = np.cos(th8[p * 128 : (p + 1) * 128])
        coef[:, 2 * p + 1] = np.sin(th8[p * 128 : (p + 1) * 128])
        coef[:, 8 + 2 * p] = np.cos(th9[p * 128 : (p + 1) * 128])
        coef[:, 9 + 2 * p] = np.sin(th9[p * 128 : (p + 1) * 128])
        coef[:, 16 + 4 + p] = -np.sin(th9[p * 128 : (p + 1) * 128])
    ident = np.eye(128, dtype=ml_dtypes.bfloat16)
    return [
        {
            "x": x[c * TOK_PER_CORE : (c + 1) * TOK_PER_CORE],
            "w": w,
            "ident": ident,
            "coef": coef,
        }
        for c in range(N_CORES)
    ]


def finalize_output(y: np.ndarray) -> np.ndarray:
    """Unshard-side dtype fixup: reference output is fp32 (the device
    writes bf16; the values are bf16-rounded in SBUF either way)."""
    return np.ascontiguousarray(y.astype(np.float32, copy=False))


def run(x: np.ndarray, angles: np.ndarray, trace: bool = False):
    """Run on 8 cores; returns (y_full, BassKernelResults)."""
    nc = _get_nc()
    in_maps = make_core_inputs(x, angles)
    res = run_bass_kernel_spmd(
        nc, in_maps, core_ids=list(range(N_CORES)), trace=trace
    )
    y = np.concatenate(
        [np.asarray(res.results[c]["y"]) for c in range(N_CORES)], axis=0
    )
    return finalize_output(y), res


def kernel(x: np.ndarray, angles: np.ndarray) -> np.ndarray:
    y, _ = run(x, angles, trace=False)
    return y
